# revision 26
# baseline (speedup 1.0000x reference)
"""Trainium2 Bass kernel for nn_FB_GCN (2x 2-layer GCN + attention fusion +
3 contrastive losses over dense NxN adjacency masks + dim-label loss).

Self-contained: host-side sharding/layout prep + an 8-core SPMD Bass/Tile
kernel. Data-parallel over node rows.

v2 design notes (vs baseline):
- degrees/ns/nd computed host-side; ns[src] folded into the one-hot scatter
  matrix S (host-prebuilt, streamed), eliminating the xs materialization pass
  and the degree matmuls/collectives.
- layer-1 gather precomputed host-side (feat rows in edge-slot order) and
  streamed sequentially; only layer-2 uses on-device dma_gather (indices
  sorted by src for DRAM locality, one call per tile).
- adjacency masks repacked host-side into contiguous [128,1024] chunks so the
  loss streaming runs at DMA line rate instead of strided 1KB lines.
- one activation table set for the whole kernel (natural_log_exp_and_others):
  tanh synthesized from exp, rsqrt from ln+exp. No ACT table swaps.
- z^T embeddings stored/all-gathered/matmul'd in fp8e4m3 (random rounding
  washes out in the final mean); sim matmuls use 1024-wide moving operands.
- loss-a streaming is issue-interleaved with graph-x layer 2 to keep all
  engines busy.
"""
import numpy as np
import ml_dtypes

BF16 = ml_dtypes.bfloat16
FP8 = ml_dtypes.float8_e4m3

# problem constants (hardcoded per contest rules)
N = 8192
E = 131072
IN, HID, OUT = 512, 512, 256
ATT_H = 16
LAM, ALPHA = 0.5, 0.1
SIGMA = 1e-10
NC_ = 8            # cores
ROWS = N // NC_    # 1024 rows per core
NT = ROWS // 128   # 8 node tiles per core
JB = 8             # adj col chunks of 1024
JW = 1024

_cache = {}


# ---------------------------------------------------------------- host prep
def _wrap_idx(idx):
    """dma_gather index layout: idx i at [i%16, i//16], replicated to 128 parts."""
    n = len(idx)
    assert n % 16 == 0
    w = np.asarray(idx, np.int16).reshape(n // 16, 16).T  # [16, n/16]
    return np.tile(w, (8, 1))  # [128, n/16]


def _prep_graph(edge_index, feat_bf):
    """Per-core arrays for one graph: pre-gathered L1 feat rows, weighted
    one-hot S blocks, wrapped gather indices (L2), nd scales."""
    src = np.asarray(edge_index[0], np.int64)
    dst = np.asarray(edge_index[1], np.int64)
    deg_out = np.bincount(src, minlength=N).astype(np.float64)
    deg_in = np.bincount(dst, minlength=N).astype(np.float64)
    ns = np.where(deg_out > 0, deg_out ** -0.5, 0.0).astype(np.float32)
    nd = np.where(deg_in > 0, deg_in ** -0.5, 0.0).astype(np.float32)

    # shard by dst core/tile, sort by src within tile
    per_core = []
    nb_d = 1
    for c in range(NC_):
        m = (dst // ROWS) == c
        es, ed = src[m], dst[m] - c * ROWS
        tiles = []
        for t in range(NT):
            tm = (ed // 128) == t
            s_t, d_t = es[tm], ed[tm] - t * 128
            o = np.argsort(s_t, kind="stable")
            tiles.append((s_t[o], d_t[o]))
            nb_d = max(nb_d, (len(s_t) + 127) // 128)
        per_core.append(tiles)

    g = dict(nb_d=nb_d, nd=nd)
    et = nb_d * 128
    g["gfeat"] = []    # [128, NT*nb_d*IN] bf16
    g["sblk"] = []     # [128, NT*nb_d*128] bf16
    g["src_idx"] = []  # [128, NT*nb_d*8] int16
    g["nd_arr"] = []   # [128, NT] f32
    for c in range(NC_):
        gf = np.zeros((NT, nb_d * 128, IN), BF16)
        sb = np.zeros((NT, nb_d * 128, 128), np.float32)
        idx_cols = []
        for t in range(NT):
            s_t, d_t = per_core[c][t]
            n_e = len(s_t)
            s_p = np.zeros(et, np.int64)
            s_p[:n_e] = s_t
            gf[t, :n_e] = feat_bf[s_t]
            sb[t, np.arange(n_e), d_t] = ns[s_t]
            idx_cols.append(_wrap_idx(s_p))
        # slot (t, b, p) = edge t-tile index b*128+p
        g["gfeat"].append(np.ascontiguousarray(
            gf.reshape(NT, nb_d, 128, IN).transpose(2, 0, 1, 3).reshape(128, -1)))
        g["sblk"].append(np.ascontiguousarray(
            sb.reshape(NT, nb_d, 128, 128).transpose(2, 0, 1, 3)
            .reshape(128, -1).astype(BF16)))
        g["src_idx"].append(np.concatenate(idx_cols, axis=1))
        g["nd_arr"].append(np.ascontiguousarray(
            nd[c * ROWS:(c + 1) * ROWS].reshape(NT, 128).T))
    return g


# ---------------------------------------------------------------- device kernel
def _build(nb_a, nb_x, debug=False):
    import concourse.bacc as bacc
    import concourse.mybir as mybir
    import concourse.tile as tile
    from concourse.dve_ops import TENSOR_TENSOR_REDUCE

    dt = mybir.dt
    AF = mybir.ActivationFunctionType
    AL = mybir.AluOpType

    nc = bacc.Bacc(None, num_devices=NC_, num_swdge_queues=2)

    # ---------------- I/O -----------------
    adj_in = {k: nc.dram_tensor(f"adj_{k}", [NT, JB, 128, JW], dt.bfloat16,
                                kind="ExternalInput")
              for k in ("label", "X", "rec")}
    gi = {}
    for gname, nb in (("a", nb_a), ("x", nb_x)):
        gi[gname] = dict(
            nb=nb,
            gfeat=nc.dram_tensor(f"gfeat_{gname}", [128, NT * nb * IN], dt.bfloat16,
                                 kind="ExternalInput"),
            sblk=nc.dram_tensor(f"sblk_{gname}", [128, NT * nb * 128], dt.bfloat16,
                                kind="ExternalInput"),
            src_idx=nc.dram_tensor(f"srcidx_{gname}", [128, NT * nb * 8], dt.int16,
                                   kind="ExternalInput"),
            nd=nc.dram_tensor(f"nd_{gname}", [128, NT], dt.float32,
                              kind="ExternalInput"),
            W0=nc.dram_tensor(f"W0{gname}", [IN, HID], dt.bfloat16, kind="ExternalInput"),
            W1=nc.dram_tensor(f"W1{gname}", [HID, OUT], dt.bfloat16, kind="ExternalInput"),
            b0=nc.dram_tensor(f"b0{gname}", [1, HID], dt.bfloat16, kind="ExternalInput"),
            b1=nc.dram_tensor(f"b1{gname}", [1, OUT], dt.bfloat16, kind="ExternalInput"),
        )
    xblk_in = nc.dram_tensor("xblk", [ROWS, IN], dt.bfloat16, kind="ExternalInput")
    csum_in = nc.dram_tensor("csum", [128, 4], dt.float32, kind="ExternalInput")
    wp1_in = nc.dram_tensor("Wp1", [OUT, ATT_H], dt.bfloat16, kind="ExternalInput")
    bp1_in = nc.dram_tensor("bp1", [1, ATT_H], dt.bfloat16, kind="ExternalInput")
    wp2_in = nc.dram_tensor("wp2", [ATT_H, 1], dt.bfloat16, kind="ExternalInput")
    idbf_in = nc.dram_tensor("idbf", [128, 128], dt.bfloat16, kind="ExternalInput")

    out_t = nc.dram_tensor("out", [128, 8], dt.float32, kind="ExternalOutput")
    if debug:
        dbg = {
            "h1w": nc.dram_tensor("dbg_h1w", [2, ROWS, OUT], dt.float32, kind="ExternalOutput"),
            "h2": nc.dram_tensor("dbg_h2", [2, ROWS, OUT], dt.float32, kind="ExternalOutput"),
            "hf": nc.dram_tensor("dbg_hf", [ROWS, OUT], dt.float32, kind="ExternalOutput"),
            "beta": nc.dram_tensor("dbg_beta", [128, 8], dt.float32, kind="ExternalOutput"),
            "pt": nc.dram_tensor("dbg_pt", [3, 2, 128, 8], dt.float32, kind="ExternalOutput"),
            "dc": nc.dram_tensor("dbg_dc", [4, 128, 256], dt.float32, kind="ExternalOutput"),
            "pt2": nc.dram_tensor("dbg_pt2", [2, 128, 8], dt.float32, kind="ExternalOutput"),
        }

    # collective buffers
    h1w_loc = {g: nc.dram_tensor(f"h1wloc_{g}", [ROWS, OUT], dt.bfloat16, kind="Internal")
               for g in ("a", "x")}
    h1w_full = {g: nc.dram_tensor(f"h1wfull_{g}", [NC_, ROWS, OUT], dt.bfloat16,
                                  kind="Internal", addr_space="Shared") for g in ("a", "x")}
    znt_loc = {e: nc.dram_tensor(f"zntloc_{e}", [2 * 128, ROWS], dt.float8e4, kind="Internal")
               for e in ("za", "zx", "zf")}
    znt_full = {e: nc.dram_tensor(f"zntfull_{e}", [NC_, 2 * 128, ROWS], dt.float8e4,
                                  kind="Internal", addr_space="Shared") for e in ("za", "zx", "zf")}
    dim_loc = nc.dram_tensor("dimloc", [4, 128, OUT], dt.float32, kind="Internal")
    dim_full = nc.dram_tensor("dimfull", [4, 128, OUT], dt.float32,
                              kind="Internal", addr_space="Shared")
    dw_dram = nc.dram_tensor("dw_dram", [ROWS], dt.float32, kind="Internal")

    RG = [list(range(NC_))]

    with tile.TileContext(nc) as tc:
        with tc.tile_pool(name="const", bufs=1) as constp, \
             tc.tile_pool(name="emb", bufs=1) as embp, \
             tc.tile_pool(name="work", bufs=2) as work, \
             tc.tile_pool(name="stat", bufs=1) as statp:

            # ---------- constants ----------
            idbf_sb = constp.tile([128, 128], dt.bfloat16)
            nc.sync.dma_start(idbf_sb[:], idbf_in[:])
            ones_row = constp.tile([1, 128], dt.bfloat16)
            nc.vector.memset(ones_row[:], 1.0)

            wp1_sb = constp.tile([128, 2, ATT_H], dt.bfloat16)
            nc.sync.dma_start(wp1_sb[:], wp1_in.rearrange("(kc p) a -> p kc a", p=128))
            bp1_sb = constp.tile([1, ATT_H], dt.bfloat16)
            nc.sync.dma_start(bp1_sb[:], bp1_in[:])
            wp2_sb = constp.tile([16, 1], dt.bfloat16)
            nc.sync.dma_start(wp2_sb[:], wp2_in[:])

            xblk_sb = constp.tile([128, NT, IN], dt.bfloat16)
            nc.sync.dma_start(xblk_sb[:], xblk_in.rearrange("(t p) f -> p t f", p=128))
            csum_sb = constp.tile([128, 4], dt.float32)
            nc.sync.dma_start(csum_sb[:], csum_in[:])

            gc = {}
            for g in ("a", "x"):
                G = gi[g]
                cs = {}
                cs["nd"] = constp.tile([128, NT], dt.float32, name=f"nd_{g}")
                nc.sync.dma_start(cs["nd"][:], G["nd"][:])
                cs["srcidx"] = constp.tile([128, NT * G["nb"] * 8], dt.int16,
                                           name=f"srcidx_{g}")
                nc.sync.dma_start(cs["srcidx"][:], G["src_idx"][:])
                cs["w0"] = constp.tile([128, 4, HID], dt.bfloat16, name=f"w0_{g}")
                nc.sync.dma_start(cs["w0"][:], G["W0"].rearrange("(kc p) f -> p kc f", p=128))
                cs["w1"] = constp.tile([128, 4, OUT], dt.bfloat16, name=f"w1_{g}")
                nc.sync.dma_start(cs["w1"][:], G["W1"].rearrange("(kc p) f -> p kc f", p=128))
                cs["b0"] = constp.tile([1, HID], dt.bfloat16, name=f"b0_{g}")
                nc.sync.dma_start(cs["b0"][:], G["b0"][:])
                cs["b1"] = constp.tile([1, OUT], dt.bfloat16, name=f"b1_{g}")
                nc.sync.dma_start(cs["b1"][:], G["b1"][:])
                gc[g] = cs

            # embedding stores
            h2_sb = {g: embp.tile([128, NT * OUT], dt.bfloat16, name=f"h2_{g}")
                     for g in ("a", "x")}
            hf_sb = embp.tile([128, NT * OUT], dt.bfloat16)
            znt_own = {e: embp.tile([128, 2, ROWS], dt.float8e4, name=f"zntown_{e}")
                       for e in ("za", "zx", "zf")}
            nrm2 = {e: statp.tile([128, NT], dt.float32, name=f"nrm2_{e}")
                    for e in ("za", "zx", "zf")}
            rsc = {e: statp.tile([128, NT], dt.float32, name=f"rsc_{e}")
                   for e in ("za", "zx", "zf")}

            loss_parts = statp.tile([128, 8], dt.float32)
            nc.vector.memset(loss_parts[:], 0.0)

            # b1 broadcast tiles
            b1_bcast = {}
            with tc.tile_pool(name="psb", bufs=1, space="PSUM") as psb:
                for g in ("a", "x"):
                    b1b_ps = psb.tile([128, OUT], dt.float32, tag="b1b", bufs=2)
                    nc.tensor.matmul(b1b_ps[:], ones_row[:], gc[g]["b1"][:],
                                     start=True, stop=True)
                    b1_bcast[g] = constp.tile([128, OUT], dt.bfloat16, name=f"b1b_{g}")
                    nc.vector.tensor_copy(b1_bcast[g][:], b1b_ps[:])

            # =======================================================
            # GCN layer-1 for both graphs (pre-gathered inputs)
            # =======================================================
            def l1_tile(g, t):
                G, cs = gi[g], gc[g]
                nb = G["nb"]
                g1 = l1p.tile([128, nb, IN], dt.bfloat16, name="g1")
                nc.sync.dma_start(
                    g1[:], G["gfeat"][:, t * nb * IN:(t + 1) * nb * IN]
                    .rearrange("p (b f) -> p b f", b=nb))
                s1 = l1p.tile([128, nb, 128], dt.bfloat16, name="s1")
                nc.sync.dma_start(
                    s1[:], G["sblk"][:, t * nb * 128:(t + 1) * nb * 128]
                    .rearrange("p (b f) -> p b f", b=nb))
                agg_ps = psg.tile([128, IN], dt.float32, name="agg_ps", tag="acc", bufs=2)
                for b in range(nb):
                    nc.tensor.matmul(agg_ps[:], s1[:, b, :], g1[:, b, :],
                                     start=(b == 0), stop=(b == nb - 1))
                aggn = work.tile([128, IN], dt.bfloat16, name="aggn")
                nc.scalar.activation(aggn[:], agg_ps[:], AF.Copy,
                                     scale=cs["nd"][:, t:t + 1])
                h1_ps = psg.tile([128, HID], dt.float32, name="h1_ps", tag="acc", bufs=2)
                for kc in range(4):
                    tr_ps = psg.tile([128, 128], dt.bfloat16, name="tr_ps", tag="tr", bufs=2)
                    nc.tensor.transpose(tr_ps[:], aggn[:, kc * 128:(kc + 1) * 128],
                                        idbf_sb[:])
                    trsb = work.tile([128, 128], dt.bfloat16, name="trsb")
                    nc.vector.tensor_copy(trsb[:], tr_ps[:])
                    nc.tensor.matmul(h1_ps[:], trsb[:], cs["w0"][:, kc, :],
                                     start=(kc == 0), stop=False)
                nc.tensor.matmul(h1_ps[:], ones_row[:], cs["b0"][:],
                                 start=False, stop=True)
                h1s = work.tile([128, HID], dt.bfloat16, name="h1s")
                nc.scalar.activation(h1s[:], h1_ps[:], AF.Relu)
                h1w_ps = psg.tile([128, OUT], dt.float32, name="h1w_ps", tag="acc", bufs=2)
                for kc in range(4):
                    tr2_ps = psg.tile([128, 128], dt.bfloat16, name="tr2_ps", tag="tr", bufs=2)
                    nc.tensor.transpose(tr2_ps[:], h1s[:, kc * 128:(kc + 1) * 128],
                                        idbf_sb[:])
                    tr2sb = work.tile([128, 128], dt.bfloat16, name="tr2sb")
                    nc.vector.tensor_copy(tr2sb[:], tr2_ps[:])
                    nc.tensor.matmul(h1w_ps[:], tr2sb[:], cs["w1"][:, kc, :],
                                     start=(kc == 0), stop=(kc == 3))
                h1w_sb = work.tile([128, OUT], dt.bfloat16, name="h1w_sb")
                nc.vector.tensor_copy(h1w_sb[:], h1w_ps[:])
                nc.sync.dma_start(h1w_loc[g][t * 128:(t + 1) * 128, :], h1w_sb[:])
                if debug:
                    ig = 0 if g == "a" else 1
                    h1wd = work.tile([128, OUT], dt.float32, name="h1wd")
                    nc.vector.tensor_copy(h1wd[:], h1w_ps[:])
                    nc.sync.dma_start(dbg["h1w"][ig, t * 128:(t + 1) * 128, :], h1wd[:])

            def l2_tile(g, t):
                G, cs = gi[g], gc[g]
                nb = G["nb"]
                h1w_view = h1w_full[g].rearrange("c r f -> (c r) f")
                g2 = l2p.tile([128, nb, OUT], dt.bfloat16, name="g2")
                for b0 in range(0, nb, 8):
                    nbc = min(8, nb - b0)
                    nc.gpsimd.dma_gather(
                        out_ap=g2[:, b0:b0 + nbc, :], in_ap=h1w_view,
                        idxs_ap=gc[g]["srcidx"][:, t * nb * 8 + b0 * 8:
                                                t * nb * 8 + (b0 + nbc) * 8],
                        num_idxs=nbc * 128, num_idxs_reg=nbc * 128, elem_size=OUT,
                        queue_num=(t + b0 // 8) % 2)
                s2 = l2p.tile([128, nb, 128], dt.bfloat16, name="s2")
                nc.sync.dma_start(
                    s2[:], G["sblk"][:, t * nb * 128:(t + 1) * nb * 128]
                    .rearrange("p (b f) -> p b f", b=nb))
                agg2_ps = psg.tile([128, OUT], dt.float32, name="agg2_ps", tag="acc", bufs=2)
                for b in range(nb):
                    nc.tensor.matmul(agg2_ps[:], s2[:, b, :], g2[:, b, :],
                                     start=(b == 0), stop=(b == nb - 1))
                h2t = work.tile([128, OUT], dt.bfloat16, name="h2t")
                nc.scalar.activation(h2t[:], agg2_ps[:], AF.Copy,
                                     scale=cs["nd"][:, t:t + 1])
                seg = h2_sb[g][:, t * OUT:(t + 1) * OUT]
                nc.vector.tensor_tensor(out=seg, in0=h2t[:], in1=b1_bcast[g][:], op=AL.add)
                # row norm^2 for l2norm
                e = "za" if g == "a" else "zx"
                scr = work.tile([128, OUT], dt.bfloat16, name="scr")
                nc.vector._custom_dve(TENSOR_TENSOR_REDUCE, out=scr[:],
                                      in0=seg, in1=seg, s0=0.0, s1=1.0,
                                      accum_out=nrm2[e][:, t:t + 1])

            def rsqrt_batch(e):
                # r = exp(-0.5*ln(max(nrm2, 1e-30)))  [128, NT]
                r = rsc[e]
                nc.vector.tensor_scalar(out=r[:], in0=nrm2[e][:], scalar1=1e-30,
                                        scalar2=None, op0=AL.max)
                nc.scalar.activation(r[:], r[:], AF.Ln)
                nc.scalar.activation(r[:], r[:], AF.Exp, scale=-0.5)

            def z_tiles(e, src_sb):
                # normalize rows, transpose to znt_own[e] (fp8)
                for t in range(NT):
                    seg = src_sb[:, t * OUT:(t + 1) * OUT]
                    zn_t = work.tile([128, OUT], dt.bfloat16, name="zn_t")
                    nc.vector.tensor_scalar(out=zn_t[:], in0=seg,
                                            scalar1=rsc[e][:, t:t + 1],
                                            scalar2=None, op0=AL.mult)
                    for kc in range(2):
                        zt_ps = psg.tile([128, 128], dt.bfloat16, name="zt_ps",
                                         tag="tr", bufs=2)
                        nc.tensor.transpose(zt_ps[:], zn_t[:, kc * 128:(kc + 1) * 128],
                                            idbf_sb[:])
                        nc.vector.tensor_copy(
                            znt_own[e][:, kc, t * 128:(t + 1) * 128], zt_ps[:])
                nc.sync.dma_start(
                    znt_loc[e].rearrange("(kc p) j -> p kc j", p=128), znt_own[e][:])
                nc.gpsimd.collective_compute(
                    "AllGather", AL.bypass, replica_groups=RG,
                    ins=[znt_loc[e][:]], outs=[znt_full[e][:]])

            # full z^T tiles [128, 2, N] fp8
            znt_sb = {e: embp.tile([128, 2, N], dt.float8e4, name=f"zntsb_{e}")
                      for e in ("za", "zx", "zf")}

            def znt_load(e):
                for c in range(NC_):
                    nc.sync.dma_start(
                        znt_sb[e][:, :, c * ROWS:(c + 1) * ROWS],
                        znt_full[e][c].rearrange("(kc p) j -> p kc j", p=128))

            # loss streaming for one (embedding, adjacency) pair, one row-tile
            loss_stats = {}
            for il, e in enumerate(("za", "zx", "zf")):
                loss_stats[e] = (statp.tile([128, NT], dt.float32, name=f"tot_{e}"),
                                 statp.tile([128, NT], dt.float32, name=f"pos_{e}"))

            def loss_tile(e, akey, t):
                tot_all, pos_all = loss_stats[e]
                tot_cols = lp.tile([128, JB], dt.float32, name="tot_cols")
                pos_cols = lp.tile([128, JB], dt.float32, name="pos_cols")
                lhs2 = znt_own[e][:, :, t * 128:(t + 1) * 128]  # [128, 2, 128] fp8
                for jb in range(JB):
                    sim_ps = psl.tile([128, JW], dt.float32, name="sim_ps",
                                      tag="sim", bufs=2)
                    for h in range(2):  # 512-wide halves: one PSUM bank per matmul
                        j0 = jb * JW + h * 512
                        # DoubleRow: virtual K=256 via fp8 pair-packing
                        nc.tensor.matmul(sim_ps[:, h * 512:(h + 1) * 512], lhs2,
                                         znt_sb[e][:, :, j0:j0 + 512],
                                         start=True, stop=True,
                                         perf_mode=mybir.MatmulPerfMode.DoubleRow)
                    refl = lp.tile([128, JW], dt.bfloat16, name="refl")
                    nc.scalar.activation(refl[:], sim_ps[:], AF.Exp,
                                         accum_out=tot_cols[:, jb:jb + 1])
                    adj_t = lp.tile([128, JW], dt.bfloat16, name="adj_t", bufs=3)
                    nc.sync.dma_start(adj_t[:], adj_in[akey][t, jb])
                    mscr = lp.tile([128, JW], dt.bfloat16, name="mscr")
                    nc.vector._custom_dve(
                        TENSOR_TENSOR_REDUCE, out=mscr[:], in0=refl[:],
                        in1=adj_t[:], s0=0.0, s1=1.0,
                        accum_out=pos_cols[:, jb:jb + 1])
                nc.vector.reduce_sum(tot_all[:, t:t + 1], tot_cols[:],
                                     axis=mybir.AxisListType.X)
                nc.vector.reduce_sum(pos_all[:, t:t + 1], pos_cols[:],
                                     axis=mybir.AxisListType.X)

            def loss_final(il, e):
                tot_all, pos_all = loss_stats[e]
                neg = lp.tile([128, NT], dt.float32, name="neg", bufs=1)
                nc.vector.tensor_tensor(out=neg[:], in0=tot_all[:], in1=pos_all[:],
                                        op=AL.subtract)
                if debug:
                    psd = work.tile([128, NT], dt.float32, name="psd")
                    nc.vector.tensor_copy(psd[:], pos_all[:])
                    nc.sync.dma_start(dbg["pt"][il, 0], psd[:])
                    ttd = work.tile([128, NT], dt.float32, name="ttd")
                    nc.vector.tensor_copy(ttd[:], tot_all[:])
                    nc.sync.dma_start(dbg["pt"][il, 1], ttd[:])
                nc.vector.tensor_scalar(out=pos_all[:], in0=pos_all[:],
                                        scalar1=SIGMA, scalar2=None, op0=AL.add)
                nc.vector.tensor_scalar(out=neg[:], in0=neg[:],
                                        scalar1=SIGMA, scalar2=None, op0=AL.add)
                nc.scalar.activation(pos_all[:], pos_all[:], AF.Ln)
                nc.scalar.activation(neg[:], neg[:], AF.Ln)
                dl = lp.tile([128, NT], dt.float32, name="dl", bufs=1)
                nc.vector.tensor_tensor(out=dl[:], in0=neg[:], in1=pos_all[:],
                                        op=AL.subtract)
                nc.vector.reduce_sum(loss_parts[:, il:il + 1], dl[:],
                                     axis=mybir.AxisListType.X)

            # =======================================================
            # issue schedule
            # =======================================================
            with tc.tile_pool(name="psg", bufs=1, space="PSUM") as psg, \
                 tc.tile_pool(name="psl", bufs=1, space="PSUM") as psl:

                # L1 both graphs; h1w all-gathers overlap the other graph
                with tc.tile_pool(name="l1p", bufs=2) as l1p:
                    for t in range(NT):
                        l1_tile("a", t)
                    nc.gpsimd.collective_compute(
                        "AllGather", AL.bypass, replica_groups=RG,
                        ins=[h1w_loc["a"][:]], outs=[h1w_full["a"][:]])
                    for t in range(NT):
                        l1_tile("x", t)
                    nc.gpsimd.collective_compute(
                        "AllGather", AL.bypass, replica_groups=RG,
                        ins=[h1w_loc["x"][:]], outs=[h1w_full["x"][:]])

                # separate scope so l1p's stream buffers are released before
                # the l2/loss pools allocate
                l2p_cm = tc.tile_pool(name="l2p", bufs=2)
                lp_cm = tc.tile_pool(name="loss", bufs=2)
                l2p = l2p_cm.__enter__()
                lp = lp_cm.__enter__()

                # L2-a -> h2a -> za -> AG
                for t in range(NT):
                    l2_tile("a", t)
                rsqrt_batch("za")
                z_tiles("za", h2_sb["a"])
                znt_load("za")

                # loss-a interleaved with L2-x
                for t in range(NT):
                    loss_tile("za", "label", t)
                    l2_tile("x", t)
                loss_final(0, "za")

                rsqrt_batch("zx")
                z_tiles("zx", h2_sb["x"])
                znt_load("zx")

                # ---------- attention fusion (tanh via exp) ----------
                w_rows = statp.tile([1, 2 * ROWS], dt.float32)  # wx | wadj
                for ib, g in enumerate(("x", "a")):
                    for t in range(NT):
                        t1_ps = psg.tile([16, 128], dt.float32, name="t1_ps",
                                         tag="acc", bufs=2)
                        for kc in range(2):
                            trh_ps = psg.tile([128, 128], dt.bfloat16, name="trh_ps",
                                              tag="tr", bufs=2)
                            nc.tensor.transpose(
                                trh_ps[:],
                                h2_sb[g][:, t * OUT + kc * 128: t * OUT + kc * 128 + 128],
                                idbf_sb[:])
                            trh = work.tile([128, 128], dt.bfloat16, name="trh")
                            nc.vector.tensor_copy(trh[:], trh_ps[:])
                            nc.tensor.matmul(t1_ps[:], wp1_sb[:, kc, :],
                                             trh[:], start=(kc == 0), stop=False)
                        nc.tensor.matmul(t1_ps[:], bp1_sb[:], ones_row[:],
                                         start=False, stop=True)
                        # tanh(u) = 1 - 2/(exp(2u)+1)
                        e2u = work.tile([16, 128], dt.float32, name="e2u")
                        nc.scalar.activation(e2u[:], t1_ps[:], AF.Exp, scale=2.0)
                        nc.vector.tensor_scalar(out=e2u[:], in0=e2u[:], scalar1=1.0,
                                                scalar2=None, op0=AL.add)
                        nc.vector.reciprocal(e2u[:], e2u[:])
                        t1_sb = work.tile([16, 128], dt.bfloat16, name="t1_sb")
                        nc.vector.tensor_scalar(out=t1_sb[:], in0=e2u[:], scalar1=-2.0,
                                                scalar2=1.0, op0=AL.mult, op1=AL.add)
                        w_ps = psg.tile([1, 128], dt.float32, name="w_ps",
                                        tag="acc", bufs=2)
                        nc.tensor.matmul(w_ps[:], wp2_sb[:], t1_sb[:], start=True, stop=True)
                        nc.vector.tensor_copy(
                            w_rows[:, ib * ROWS + t * 128: ib * ROWS + (t + 1) * 128],
                            w_ps[:])
                # beta_x = sigmoid(wx - wadj) on [1, 1024]
                dw = statp.tile([1, ROWS], dt.float32)
                nc.vector.tensor_tensor(out=dw[:], in0=w_rows[:, 0:ROWS],
                                        in1=w_rows[:, ROWS:2 * ROWS], op=AL.subtract)
                nc.scalar.activation(dw[:], dw[:], AF.Exp, scale=-1.0)
                nc.vector.tensor_scalar(out=dw[:], in0=dw[:], scalar1=1.0,
                                        scalar2=None, op0=AL.add)
                nc.vector.reciprocal(dw[:], dw[:])
                nc.sync.dma_start(dw_dram.rearrange("(o x) -> o x", o=1), dw[:])
                beta_col = statp.tile([128, 1, NT], dt.float32)
                nc.sync.dma_start(beta_col[:],
                                  dw_dram.rearrange("(t p o) -> p o t", p=128, o=1))
                if debug:
                    nc.sync.dma_start(dbg["beta"][:], beta_col[:, 0, :])
                # h_fuse = h_adj + beta*(h_x - h_adj); row norm^2 on the fly
                from concourse.dve_ops import TENSOR_TENSOR_REDUCE as TTR
                for t in range(NT):
                    dhf = work.tile([128, OUT], dt.bfloat16, name="dhf")
                    nc.vector.tensor_tensor(out=dhf[:], in0=h2_sb["x"][:, t * OUT:(t + 1) * OUT],
                                            in1=h2_sb["a"][:, t * OUT:(t + 1) * OUT],
                                            op=AL.subtract)
                    seg = hf_sb[:, t * OUT:(t + 1) * OUT]
                    nc.vector.scalar_tensor_tensor(
                        out=seg, in0=dhf[:],
                        scalar=beta_col[:, 0, t:t + 1],
                        in1=h2_sb["a"][:, t * OUT:(t + 1) * OUT],
                        op0=AL.mult, op1=AL.add)
                    scr = work.tile([128, OUT], dt.bfloat16, name="scr")
                    nc.vector._custom_dve(TTR, out=scr[:], in0=seg, in1=seg,
                                          s0=0.0, s1=1.0,
                                          accum_out=nrm2["zf"][:, t:t + 1])
                if debug:
                    for t in range(NT):
                        hfd = work.tile([128, OUT], dt.float32, name="hfd")
                        nc.vector.tensor_copy(hfd[:], hf_sb[:, t * OUT:(t + 1) * OUT])
                        nc.sync.dma_start(dbg["hf"][t * 128:(t + 1) * 128, :], hfd[:])

                rsqrt_batch("zf")
                z_tiles("zf", hf_sb)
                znt_load("zf")

                # ---------- dim partials + AllReduce (overlaps loss-x) ----------
                # colsum(X) is input-static: computed host-side (csum_in).
                hfb = lp.tile([128, NT, OUT], dt.bfloat16, bufs=1)
                for t in range(NT):
                    nc.vector.tensor_copy(hfb[:, t, :], hf_sb[:, t * OUT:(t + 1) * OUT])
                dim_sb = lp.tile([128, 4, OUT], dt.float32, bufs=1)
                for mt in range(4):
                    xtz_ps = psg.tile([128, OUT], dt.float32, name="xtz_ps",
                                      tag="acc", bufs=2)
                    for t in range(NT):
                        nc.tensor.matmul(xtz_ps[:],
                                         xblk_sb[:, t, mt * 128:(mt + 1) * 128],
                                         hfb[:, t, :], start=(t == 0), stop=(t == NT - 1))
                    nc.vector.tensor_copy(dim_sb[:, mt, :], xtz_ps[:])
                nc.sync.dma_start(dim_loc.rearrange("m p f -> p m f"), dim_sb[:])
                nc.gpsimd.collective_compute(
                    "AllReduce", AL.add, replica_groups=RG,
                    ins=[dim_loc[:]], outs=[dim_full[:]])

                # ---------- loss-x ----------
                for t in range(NT):
                    loss_tile("zx", "X", t)
                loss_final(1, "zx")

                # ---------- dim centers ----------
                dimf = lp.tile([128, 4, OUT], dt.float32, bufs=1)
                nc.sync.dma_start(dimf[:], dim_full.rearrange("m p f -> p m f"))
                dcnT = lp.tile([128, 2, 512], dt.float8e4, bufs=1)
                dcn2 = lp.tile([128, 4], dt.float32, bufs=1)
                dc_store = lp.tile([128, 4, OUT], dt.bfloat16, bufs=1)
                for mt in range(4):
                    nc.vector.tensor_scalar(out=dc_store[:, mt, :], in0=dimf[:, mt, :],
                                            scalar1=csum_sb[:, mt:mt + 1],
                                            scalar2=None, op0=AL.mult)
                    if debug:
                        dcd = work.tile([128, OUT], dt.float32, name="dcd")
                        nc.vector.tensor_copy(dcd[:], dc_store[:, mt, :])
                        nc.sync.dma_start(dbg["dc"][mt], dcd[:])
                    scr = work.tile([128, OUT], dt.bfloat16, name="scrd")
                    nc.vector._custom_dve(TENSOR_TENSOR_REDUCE, out=scr[:],
                                          in0=dc_store[:, mt, :], in1=dc_store[:, mt, :],
                                          s0=0.0, s1=1.0, accum_out=dcn2[:, mt:mt + 1])
                nc.vector.tensor_scalar(out=dcn2[:], in0=dcn2[:], scalar1=1e-30,
                                        scalar2=None, op0=AL.max)
                nc.scalar.activation(dcn2[:], dcn2[:], AF.Ln)
                nc.scalar.activation(dcn2[:], dcn2[:], AF.Exp, scale=-0.5)
                for mt in range(4):
                    dcn_t = work.tile([128, OUT], dt.bfloat16, name="dcn_t")
                    nc.vector.tensor_scalar(out=dcn_t[:], in0=dc_store[:, mt, :],
                                            scalar1=dcn2[:, mt:mt + 1],
                                            scalar2=None, op0=AL.mult)
                    for kc in range(2):
                        dct_ps = psg.tile([128, 128], dt.bfloat16, name="dct_ps",
                                          tag="tr", bufs=2)
                        nc.tensor.transpose(dct_ps[:], dcn_t[:, kc * 128:(kc + 1) * 128],
                                            idbf_sb[:])
                        nc.vector.tensor_copy(dcnT[:, kc, mt * 128:(mt + 1) * 128],
                                              dct_ps[:])

                # ---------- loss-f ----------
                for t in range(NT):
                    loss_tile("zf", "rec", t)
                loss_final(2, "zf")

                # ---------- dim-label loss ----------
                tot2 = lp.tile([128, NT], dt.float32, bufs=1)
                pos2 = lp.tile([128, NT], dt.float32, bufs=1)
                for t in range(NT):
                    r2_ps = psl.tile([128, 512], dt.float32, name="r2_ps",
                                     tag="sim", bufs=2)
                    nc.tensor.matmul(r2_ps[:], znt_own["zf"][:, :, t * 128:(t + 1) * 128],
                                     dcnT[:, :, :], start=True, stop=True,
                                     perf_mode=mybir.MatmulPerfMode.DoubleRow)
                    refl2 = lp.tile([128, 512], dt.bfloat16, name="refl2")
                    nc.scalar.activation(refl2[:], r2_ps[:], AF.Exp,
                                         accum_out=tot2[:, t:t + 1])
                    xhot = lp.tile([128, 512], dt.bfloat16, name="xhot")
                    nc.vector.tensor_scalar(out=xhot[:], in0=xblk_sb[:, t, :],
                                            scalar1=0.0, scalar2=None, op0=AL.is_gt)
                    scr2 = lp.tile([128, 512], dt.bfloat16, name="scr2")
                    nc.vector._custom_dve(TENSOR_TENSOR_REDUCE, out=scr2[:],
                                          in0=refl2[:], in1=xhot[:], s0=0.0, s1=1.0,
                                          accum_out=pos2[:, t:t + 1])
                if debug:
                    p2d = work.tile([128, NT], dt.float32, name="p2d")
                    nc.vector.tensor_copy(p2d[:], pos2[:])
                    nc.sync.dma_start(dbg["pt2"][0], p2d[:])
                    t2d = work.tile([128, NT], dt.float32, name="t2d")
                    nc.vector.tensor_copy(t2d[:], tot2[:])
                    nc.sync.dma_start(dbg["pt2"][1], t2d[:])
                # loss_feat partial: -ln(pos/neg + 1e-5), pos=pos2+SIG, neg=tot2-pos2
                neg2 = lp.tile([128, NT], dt.float32, bufs=1)
                nc.vector.tensor_tensor(out=neg2[:], in0=tot2[:], in1=pos2[:],
                                        op=AL.subtract)
                nc.vector.tensor_scalar(out=pos2[:], in0=pos2[:], scalar1=SIGMA,
                                        scalar2=None, op0=AL.add)
                nc.vector.reciprocal(neg2[:], neg2[:])
                r = lp.tile([128, NT], dt.float32, bufs=1)
                nc.vector.tensor_tensor(out=r[:], in0=pos2[:], in1=neg2[:], op=AL.mult)
                nc.vector.tensor_scalar(out=r[:], in0=r[:], scalar1=1e-5,
                                        scalar2=None, op0=AL.add)
                nc.scalar.activation(r[:], r[:], AF.Ln)
                rsum = lp.tile([128, 1], dt.float32, bufs=1)
                nc.vector.reduce_sum(rsum[:], r[:], axis=mybir.AxisListType.X)
                nc.vector.tensor_scalar(out=loss_parts[:, 3:4], in0=rsum[:],
                                        scalar1=-1.0, scalar2=None, op0=AL.mult)

                lp_cm.__exit__(None, None, None)
                l2p_cm.__exit__(None, None, None)

            # ---------- output ----------
            nc.sync.dma_start(out_t[:], loss_parts[:])

    nc.compile()
    return nc


# ---------------------------------------------------------------- entry point
def _prep(feat, adj_label, adj_X, adj_rec, W0a, b0a, W1a, b1a,
          W0x, b0x, W1x, b1x, Wp1, bp1, wp2, edge_index, edge_index_x,
          _debug=False):
    feat = np.asarray(feat, np.float32)
    feat_bf = feat.astype(BF16)
    ga = _prep_graph(np.asarray(edge_index), feat_bf)
    gx = _prep_graph(np.asarray(edge_index_x), feat_bf)

    key = (ga["nb_d"], gx["nb_d"], _debug)
    if key not in _cache:
        _cache[key] = _build(*key[:2], debug=_debug)
    nc = _cache[key]

    idbf = np.eye(128, dtype=np.float32).astype(BF16)

    colsum = feat.sum(axis=0)  # [IN]
    crecip = (1.0 / (colsum + 1e-5)).astype(np.float32).reshape(4, 128).T

    base = dict(
        idbf=idbf,
        csum=np.ascontiguousarray(crecip),
        W0a=np.asarray(W0a, np.float32).astype(BF16),
        W1a=np.asarray(W1a, np.float32).astype(BF16),
        b0a=np.asarray(b0a, np.float32).reshape(1, HID).astype(BF16),
        b1a=np.asarray(b1a, np.float32).reshape(1, OUT).astype(BF16),
        W0x=np.asarray(W0x, np.float32).astype(BF16),
        W1x=np.asarray(W1x, np.float32).astype(BF16),
        b0x=np.asarray(b0x, np.float32).reshape(1, HID).astype(BF16),
        b1x=np.asarray(b1x, np.float32).reshape(1, OUT).astype(BF16),
        Wp1=np.asarray(Wp1, np.float32).astype(BF16),
        bp1=np.asarray(bp1, np.float32).reshape(1, ATT_H).astype(BF16),
        wp2=np.asarray(wp2, np.float32).astype(BF16),
    )
    adj_bf = {k: np.asarray(v, np.float32).astype(BF16)
              for k, v in (("label", adj_label), ("X", adj_X), ("rec", adj_rec))}

    in_maps = []
    for c in range(NC_):
        m = dict(base)
        m["xblk"] = feat_bf[c * ROWS:(c + 1) * ROWS]
        for k in ("label", "X", "rec"):
            blk = adj_bf[k][c * ROWS:(c + 1) * ROWS]  # [1024, 8192]
            m[f"adj_{k}"] = np.ascontiguousarray(
                blk.reshape(NT, 128, JB, JW).transpose(0, 2, 1, 3))
        for gname, g in (("a", ga), ("x", gx)):
            m[f"gfeat_{gname}"] = g["gfeat"][c]
            m[f"sblk_{gname}"] = g["sblk"][c]
            m[f"srcidx_{gname}"] = np.ascontiguousarray(g["src_idx"][c])
            m[f"nd_{gname}"] = g["nd_arr"][c]
        in_maps.append(m)

    return nc, in_maps


def kernel(_debug=False, _trace=False, **inputs):
    from concourse.bass_utils import run_bass_kernel_spmd
    nc, in_maps = _prep(_debug=_debug, **inputs)
    res = run_bass_kernel_spmd(nc, in_maps, core_ids=list(range(NC_)), trace=_trace)
    parts = np.stack([r["out"] for r in res.results])  # [8, 128, 8]
    psum = parts.sum(axis=(0, 1))  # [8]
    la, lx, ladj, lf = psum[0] / N, psum[1] / N, psum[2] / N, psum[3] / N
    val = np.float32(LAM * (la + lx) + ALPHA * lf + ladj)
    if _debug or _trace:
        kernel._last = res
    return np.asarray(val, np.float32).reshape(())


# revision 27
# speedup vs baseline: 1.0241x; 1.0241x over previous
"""Trainium2 Bass kernel for nn_FB_GCN (2x 2-layer GCN + attention fusion +
3 contrastive losses over dense NxN adjacency masks + dim-label loss).

Self-contained: host-side sharding/layout prep + an 8-core SPMD Bass/Tile
kernel. Data-parallel over node rows.

v2 design notes (vs baseline):
- degrees/ns/nd computed host-side; ns[src] folded into the one-hot scatter
  matrix S (host-prebuilt, streamed), eliminating the xs materialization pass
  and the degree matmuls/collectives.
- layer-1 gather precomputed host-side (feat rows in edge-slot order) and
  streamed sequentially; only layer-2 uses on-device dma_gather (indices
  sorted by src for DRAM locality, one call per tile).
- adjacency masks repacked host-side into contiguous [128,1024] chunks so the
  loss streaming runs at DMA line rate instead of strided 1KB lines.
- one activation table set for the whole kernel (natural_log_exp_and_others):
  tanh synthesized from exp, rsqrt from ln+exp. No ACT table swaps.
- z^T embeddings stored/all-gathered/matmul'd in fp8e4m3 (random rounding
  washes out in the final mean); sim matmuls use 1024-wide moving operands.
- loss-a streaming is issue-interleaved with graph-x layer 2 to keep all
  engines busy.
"""
import numpy as np
import ml_dtypes

BF16 = ml_dtypes.bfloat16
FP8 = ml_dtypes.float8_e4m3

# problem constants (hardcoded per contest rules)
N = 8192
E = 131072
IN, HID, OUT = 512, 512, 256
ATT_H = 16
LAM, ALPHA = 0.5, 0.1
SIGMA = 1e-10
NC_ = 8            # cores
ROWS = N // NC_    # 1024 rows per core
NT = ROWS // 128   # 8 node tiles per core
JB = 8             # adj col chunks of 1024
JW = 1024

_cache = {}


# ---------------------------------------------------------------- host prep
def _wrap_idx(idx):
    """dma_gather index layout: idx i at [i%16, i//16], replicated to 128 parts."""
    n = len(idx)
    assert n % 16 == 0
    w = np.asarray(idx, np.int16).reshape(n // 16, 16).T  # [16, n/16]
    return np.tile(w, (8, 1))  # [128, n/16]


def _prep_graph(edge_index, feat_bf):
    """Per-core arrays for one graph: pre-gathered L1 feat rows, weighted
    one-hot S blocks, wrapped gather indices (L2), nd scales."""
    src = np.asarray(edge_index[0], np.int64)
    dst = np.asarray(edge_index[1], np.int64)
    deg_out = np.bincount(src, minlength=N).astype(np.float64)
    deg_in = np.bincount(dst, minlength=N).astype(np.float64)
    ns = np.where(deg_out > 0, deg_out ** -0.5, 0.0).astype(np.float32)
    nd = np.where(deg_in > 0, deg_in ** -0.5, 0.0).astype(np.float32)

    # shard by dst core/tile, sort by src within tile
    per_core = []
    nb_d = 1
    for c in range(NC_):
        m = (dst // ROWS) == c
        es, ed = src[m], dst[m] - c * ROWS
        tiles = []
        for t in range(NT):
            tm = (ed // 128) == t
            s_t, d_t = es[tm], ed[tm] - t * 128
            o = np.argsort(s_t, kind="stable")
            tiles.append((s_t[o], d_t[o]))
            nb_d = max(nb_d, (len(s_t) + 127) // 128)
        per_core.append(tiles)

    g = dict(nb_d=nb_d, nd=nd)
    et = nb_d * 128
    g["gfeat"] = []    # [128, NT*nb_d*IN] bf16
    g["sblk"] = []     # [128, NT*nb_d*128] bf16
    g["src_idx"] = []  # [128, NT*nb_d*8] int16
    g["nd_arr"] = []   # [128, NT] f32
    for c in range(NC_):
        gf = np.zeros((NT, nb_d * 128, IN), BF16)
        sb = np.zeros((NT, nb_d * 128, 128), np.float32)
        idx_cols = []
        for t in range(NT):
            s_t, d_t = per_core[c][t]
            n_e = len(s_t)
            s_p = np.zeros(et, np.int64)
            s_p[:n_e] = s_t
            gf[t, :n_e] = feat_bf[s_t]
            sb[t, np.arange(n_e), d_t] = ns[s_t]
            idx_cols.append(_wrap_idx(s_p))
        # slot (t, b, p) = edge t-tile index b*128+p
        g["gfeat"].append(np.ascontiguousarray(
            gf.reshape(NT, nb_d, 128, IN).transpose(2, 0, 1, 3).reshape(128, -1)))
        g["sblk"].append(np.ascontiguousarray(
            sb.reshape(NT, nb_d, 128, 128).transpose(2, 0, 1, 3)
            .reshape(128, -1).astype(BF16)))
        g["src_idx"].append(np.concatenate(idx_cols, axis=1))
        g["nd_arr"].append(np.ascontiguousarray(
            nd[c * ROWS:(c + 1) * ROWS].reshape(NT, 128).T))
    return g


# ---------------------------------------------------------------- device kernel
def _build(nb_a, nb_x, debug=False):
    import concourse.bacc as bacc
    import concourse.mybir as mybir
    import concourse.tile as tile
    from concourse.dve_ops import TENSOR_TENSOR_REDUCE

    dt = mybir.dt
    AF = mybir.ActivationFunctionType
    AL = mybir.AluOpType

    nc = bacc.Bacc(None, num_devices=NC_, num_swdge_queues=2)

    # ---------------- I/O -----------------
    adj_in = {k: nc.dram_tensor(f"adj_{k}", [NT, JB, 128, JW], dt.bfloat16,
                                kind="ExternalInput")
              for k in ("label", "X", "rec")}
    gi = {}
    for gname, nb in (("a", nb_a), ("x", nb_x)):
        gi[gname] = dict(
            nb=nb,
            gfeat=nc.dram_tensor(f"gfeat_{gname}", [128, NT * nb * IN], dt.bfloat16,
                                 kind="ExternalInput"),
            sblk=nc.dram_tensor(f"sblk_{gname}", [128, NT * nb * 128], dt.bfloat16,
                                kind="ExternalInput"),
            src_idx=nc.dram_tensor(f"srcidx_{gname}", [128, NT * nb * 8], dt.int16,
                                   kind="ExternalInput"),
            nd=nc.dram_tensor(f"nd_{gname}", [128, NT], dt.float32,
                              kind="ExternalInput"),
            W0=nc.dram_tensor(f"W0{gname}", [IN, HID], dt.bfloat16, kind="ExternalInput"),
            W1=nc.dram_tensor(f"W1{gname}", [HID, OUT], dt.bfloat16, kind="ExternalInput"),
            b0=nc.dram_tensor(f"b0{gname}", [1, HID], dt.bfloat16, kind="ExternalInput"),
            b1=nc.dram_tensor(f"b1{gname}", [1, OUT], dt.bfloat16, kind="ExternalInput"),
        )
    xblk_in = nc.dram_tensor("xblk", [ROWS, IN], dt.bfloat16, kind="ExternalInput")
    csum_in = nc.dram_tensor("csum", [128, 4], dt.float32, kind="ExternalInput")
    wp1_in = nc.dram_tensor("Wp1", [OUT, ATT_H], dt.bfloat16, kind="ExternalInput")
    bp1_in = nc.dram_tensor("bp1", [1, ATT_H], dt.bfloat16, kind="ExternalInput")
    wp2_in = nc.dram_tensor("wp2", [ATT_H, 1], dt.bfloat16, kind="ExternalInput")
    idbf_in = nc.dram_tensor("idbf", [128, 128], dt.bfloat16, kind="ExternalInput")

    out_t = nc.dram_tensor("out", [128, 8], dt.float32, kind="ExternalOutput")
    if debug:
        dbg = {
            "h1w": nc.dram_tensor("dbg_h1w", [2, ROWS, OUT], dt.float32, kind="ExternalOutput"),
            "h2": nc.dram_tensor("dbg_h2", [2, ROWS, OUT], dt.float32, kind="ExternalOutput"),
            "hf": nc.dram_tensor("dbg_hf", [ROWS, OUT], dt.float32, kind="ExternalOutput"),
            "beta": nc.dram_tensor("dbg_beta", [128, 8], dt.float32, kind="ExternalOutput"),
            "pt": nc.dram_tensor("dbg_pt", [3, 2, 128, 8], dt.float32, kind="ExternalOutput"),
            "dc": nc.dram_tensor("dbg_dc", [4, 128, 256], dt.float32, kind="ExternalOutput"),
            "pt2": nc.dram_tensor("dbg_pt2", [2, 128, 8], dt.float32, kind="ExternalOutput"),
        }

    # collective buffers
    h1w_loc = {g: nc.dram_tensor(f"h1wloc_{g}", [ROWS, OUT], dt.bfloat16, kind="Internal")
               for g in ("a", "x")}
    h1w_full = {g: nc.dram_tensor(f"h1wfull_{g}", [NC_, ROWS, OUT], dt.bfloat16,
                                  kind="Internal", addr_space="Shared") for g in ("a", "x")}
    znt_loc = {e: nc.dram_tensor(f"zntloc_{e}", [2 * 128, ROWS], dt.float8e4, kind="Internal")
               for e in ("za", "zx", "zf")}
    znt_full = {e: nc.dram_tensor(f"zntfull_{e}", [NC_, 2 * 128, ROWS], dt.float8e4,
                                  kind="Internal", addr_space="Shared") for e in ("za", "zx", "zf")}
    dim_loc = nc.dram_tensor("dimloc", [4, 128, OUT], dt.float32, kind="Internal")
    dim_full = nc.dram_tensor("dimfull", [4, 128, OUT], dt.float32,
                              kind="Internal", addr_space="Shared")
    dw_dram = nc.dram_tensor("dw_dram", [ROWS], dt.float32, kind="Internal")

    RG = [list(range(NC_))]

    with tile.TileContext(nc) as tc:
        with tc.tile_pool(name="const", bufs=1) as constp, \
             tc.tile_pool(name="emb", bufs=1) as embp, \
             tc.tile_pool(name="work", bufs=2) as work, \
             tc.tile_pool(name="stat", bufs=1) as statp:

            # ---------- constants ----------
            idbf_sb = constp.tile([128, 128], dt.bfloat16)
            nc.sync.dma_start(idbf_sb[:], idbf_in[:])
            ones_row = constp.tile([1, 128], dt.bfloat16)
            nc.vector.memset(ones_row[:], 1.0)

            wp1_sb = constp.tile([128, 2, ATT_H], dt.bfloat16)
            nc.sync.dma_start(wp1_sb[:], wp1_in.rearrange("(kc p) a -> p kc a", p=128))
            bp1_sb = constp.tile([1, ATT_H], dt.bfloat16)
            nc.sync.dma_start(bp1_sb[:], bp1_in[:])
            wp2_sb = constp.tile([16, 1], dt.bfloat16)
            nc.sync.dma_start(wp2_sb[:], wp2_in[:])

            xblk_sb = constp.tile([128, NT, IN], dt.bfloat16)
            nc.sync.dma_start(xblk_sb[:], xblk_in.rearrange("(t p) f -> p t f", p=128))
            csum_sb = constp.tile([128, 4], dt.float32)
            nc.sync.dma_start(csum_sb[:], csum_in[:])

            gc = {}
            for g in ("a", "x"):
                G = gi[g]
                cs = {}
                cs["nd"] = constp.tile([128, NT], dt.float32, name=f"nd_{g}")
                nc.sync.dma_start(cs["nd"][:], G["nd"][:])
                cs["srcidx"] = constp.tile([128, NT * G["nb"] * 8], dt.int16,
                                           name=f"srcidx_{g}")
                nc.sync.dma_start(cs["srcidx"][:], G["src_idx"][:])
                cs["w0"] = constp.tile([128, 4, HID], dt.bfloat16, name=f"w0_{g}")
                nc.sync.dma_start(cs["w0"][:], G["W0"].rearrange("(kc p) f -> p kc f", p=128))
                cs["w1"] = constp.tile([128, 4, OUT], dt.bfloat16, name=f"w1_{g}")
                nc.sync.dma_start(cs["w1"][:], G["W1"].rearrange("(kc p) f -> p kc f", p=128))
                cs["b0"] = constp.tile([1, HID], dt.bfloat16, name=f"b0_{g}")
                nc.sync.dma_start(cs["b0"][:], G["b0"][:])
                cs["b1"] = constp.tile([1, OUT], dt.bfloat16, name=f"b1_{g}")
                nc.sync.dma_start(cs["b1"][:], G["b1"][:])
                gc[g] = cs

            # embedding stores
            h2_sb = {g: embp.tile([128, NT * OUT], dt.bfloat16, name=f"h2_{g}")
                     for g in ("a", "x")}
            hf_sb = embp.tile([128, NT * OUT], dt.bfloat16)
            znt_own = {e: embp.tile([128, 2, ROWS], dt.float8e4, name=f"zntown_{e}")
                       for e in ("za", "zx", "zf")}
            nrm2 = {e: statp.tile([128, NT], dt.float32, name=f"nrm2_{e}")
                    for e in ("za", "zx", "zf")}
            rsc = {e: statp.tile([128, NT], dt.float32, name=f"rsc_{e}")
                   for e in ("za", "zx", "zf")}

            loss_parts = statp.tile([128, 8], dt.float32)
            nc.vector.memset(loss_parts[:], 0.0)

            # b1 broadcast tiles
            b1_bcast = {}
            with tc.tile_pool(name="psb", bufs=1, space="PSUM") as psb:
                for g in ("a", "x"):
                    b1b_ps = psb.tile([128, OUT], dt.float32, tag="b1b", bufs=2)
                    nc.tensor.matmul(b1b_ps[:], ones_row[:], gc[g]["b1"][:],
                                     start=True, stop=True)
                    b1_bcast[g] = constp.tile([128, OUT], dt.bfloat16, name=f"b1b_{g}")
                    nc.vector.tensor_copy(b1_bcast[g][:], b1b_ps[:])

            # =======================================================
            # GCN layer-1 for both graphs (pre-gathered inputs)
            # =======================================================
            def l1_tile(g, t):
                G, cs = gi[g], gc[g]
                nb = G["nb"]
                g1 = l1p.tile([128, nb, IN], dt.bfloat16, name="g1")
                nc.sync.dma_start(
                    g1[:], G["gfeat"][:, t * nb * IN:(t + 1) * nb * IN]
                    .rearrange("p (b f) -> p b f", b=nb))
                s1 = l1p.tile([128, nb, 128], dt.bfloat16, name="s1")
                nc.sync.dma_start(
                    s1[:], G["sblk"][:, t * nb * 128:(t + 1) * nb * 128]
                    .rearrange("p (b f) -> p b f", b=nb))
                agg_ps = psg.tile([128, IN], dt.float32, name="agg_ps", tag="acc", bufs=2)
                for b in range(nb):
                    nc.tensor.matmul(agg_ps[:], s1[:, b, :], g1[:, b, :],
                                     start=(b == 0), stop=(b == nb - 1))
                aggn = work.tile([128, IN], dt.bfloat16, name="aggn")
                nc.scalar.activation(aggn[:], agg_ps[:], AF.Copy,
                                     scale=cs["nd"][:, t:t + 1])
                h1_ps = psg.tile([128, HID], dt.float32, name="h1_ps", tag="acc", bufs=2)
                for kc in range(4):
                    tr_ps = psg.tile([128, 128], dt.bfloat16, name="tr_ps", tag="tr", bufs=2)
                    nc.tensor.transpose(tr_ps[:], aggn[:, kc * 128:(kc + 1) * 128],
                                        idbf_sb[:])
                    trsb = work.tile([128, 128], dt.bfloat16, name="trsb")
                    nc.vector.tensor_copy(trsb[:], tr_ps[:])
                    nc.tensor.matmul(h1_ps[:], trsb[:], cs["w0"][:, kc, :],
                                     start=(kc == 0), stop=False)
                nc.tensor.matmul(h1_ps[:], ones_row[:], cs["b0"][:],
                                 start=False, stop=True)
                h1s = work.tile([128, HID], dt.bfloat16, name="h1s")
                nc.scalar.activation(h1s[:], h1_ps[:], AF.Relu)
                h1w_ps = psg.tile([128, OUT], dt.float32, name="h1w_ps", tag="acc", bufs=2)
                for kc in range(4):
                    tr2_ps = psg.tile([128, 128], dt.bfloat16, name="tr2_ps", tag="tr", bufs=2)
                    nc.tensor.transpose(tr2_ps[:], h1s[:, kc * 128:(kc + 1) * 128],
                                        idbf_sb[:])
                    tr2sb = work.tile([128, 128], dt.bfloat16, name="tr2sb")
                    nc.vector.tensor_copy(tr2sb[:], tr2_ps[:])
                    nc.tensor.matmul(h1w_ps[:], tr2sb[:], cs["w1"][:, kc, :],
                                     start=(kc == 0), stop=(kc == 3))
                h1w_sb = work.tile([128, OUT], dt.bfloat16, name="h1w_sb")
                nc.vector.tensor_copy(h1w_sb[:], h1w_ps[:])
                nc.sync.dma_start(h1w_loc[g][t * 128:(t + 1) * 128, :], h1w_sb[:])
                if debug:
                    ig = 0 if g == "a" else 1
                    h1wd = work.tile([128, OUT], dt.float32, name="h1wd")
                    nc.vector.tensor_copy(h1wd[:], h1w_ps[:])
                    nc.sync.dma_start(dbg["h1w"][ig, t * 128:(t + 1) * 128, :], h1wd[:])

            def l2_tile(g, t):
                G, cs = gi[g], gc[g]
                nb = G["nb"]
                h1w_view = h1w_full[g].rearrange("c r f -> (c r) f")
                g2 = l2p.tile([128, nb, OUT], dt.bfloat16, name="g2", bufs=3)
                for b0 in range(0, nb, 8):
                    nbc = min(8, nb - b0)
                    nc.gpsimd.dma_gather(
                        out_ap=g2[:, b0:b0 + nbc, :], in_ap=h1w_view,
                        idxs_ap=gc[g]["srcidx"][:, t * nb * 8 + b0 * 8:
                                                t * nb * 8 + (b0 + nbc) * 8],
                        num_idxs=nbc * 128, num_idxs_reg=nbc * 128, elem_size=OUT,
                        queue_num=(t + b0 // 8) % 2)
                s2 = l2p.tile([128, nb, 128], dt.bfloat16, name="s2", bufs=3)
                nc.sync.dma_start(
                    s2[:], G["sblk"][:, t * nb * 128:(t + 1) * nb * 128]
                    .rearrange("p (b f) -> p b f", b=nb))
                agg2_ps = psg.tile([128, OUT], dt.float32, name="agg2_ps", tag="acc", bufs=2)
                for b in range(nb):
                    nc.tensor.matmul(agg2_ps[:], s2[:, b, :], g2[:, b, :],
                                     start=(b == 0), stop=(b == nb - 1))
                h2t = work.tile([128, OUT], dt.bfloat16, name="h2t")
                nc.scalar.activation(h2t[:], agg2_ps[:], AF.Copy,
                                     scale=cs["nd"][:, t:t + 1])
                seg = h2_sb[g][:, t * OUT:(t + 1) * OUT]
                nc.vector.tensor_tensor(out=seg, in0=h2t[:], in1=b1_bcast[g][:], op=AL.add)
                # row norm^2 for l2norm
                e = "za" if g == "a" else "zx"
                scr = work.tile([128, OUT], dt.bfloat16, name="scr")
                nc.vector._custom_dve(TENSOR_TENSOR_REDUCE, out=scr[:],
                                      in0=seg, in1=seg, s0=0.0, s1=1.0,
                                      accum_out=nrm2[e][:, t:t + 1])

            def rsqrt_batch(e):
                # r = exp(-0.5*ln(max(nrm2, 1e-30)))  [128, NT]
                r = rsc[e]
                nc.vector.tensor_scalar(out=r[:], in0=nrm2[e][:], scalar1=1e-30,
                                        scalar2=None, op0=AL.max)
                nc.scalar.activation(r[:], r[:], AF.Ln)
                nc.scalar.activation(r[:], r[:], AF.Exp, scale=-0.5)

            def z_tiles(e, src_sb):
                # normalize rows, transpose to znt_own[e] (fp8)
                for t in range(NT):
                    seg = src_sb[:, t * OUT:(t + 1) * OUT]
                    zn_t = work.tile([128, OUT], dt.bfloat16, name="zn_t")
                    nc.vector.tensor_scalar(out=zn_t[:], in0=seg,
                                            scalar1=rsc[e][:, t:t + 1],
                                            scalar2=None, op0=AL.mult)
                    for kc in range(2):
                        zt_ps = psg.tile([128, 128], dt.bfloat16, name="zt_ps",
                                         tag="tr", bufs=2)
                        nc.tensor.transpose(zt_ps[:], zn_t[:, kc * 128:(kc + 1) * 128],
                                            idbf_sb[:])
                        nc.vector.tensor_copy(
                            znt_own[e][:, kc, t * 128:(t + 1) * 128], zt_ps[:])
                nc.sync.dma_start(
                    znt_loc[e].rearrange("(kc p) j -> p kc j", p=128), znt_own[e][:])
                nc.gpsimd.collective_compute(
                    "AllGather", AL.bypass, replica_groups=RG,
                    ins=[znt_loc[e][:]], outs=[znt_full[e][:]])

            # full z^T tiles [128, 2, N] fp8
            znt_sb = {e: embp.tile([128, 2, N], dt.float8e4, name=f"zntsb_{e}")
                      for e in ("za", "zx", "zf")}

            def znt_load(e):
                for c in range(NC_):
                    nc.sync.dma_start(
                        znt_sb[e][:, :, c * ROWS:(c + 1) * ROWS],
                        znt_full[e][c].rearrange("(kc p) j -> p kc j", p=128))

            # loss streaming for one (embedding, adjacency) pair, one row-tile
            loss_stats = {}
            for il, e in enumerate(("za", "zx", "zf")):
                loss_stats[e] = (statp.tile([128, NT], dt.float32, name=f"tot_{e}"),
                                 statp.tile([128, NT], dt.float32, name=f"pos_{e}"))

            def loss_tile(e, akey, t):
                tot_all, pos_all = loss_stats[e]
                tot_cols = lp.tile([128, JB], dt.float32, name="tot_cols")
                pos_cols = lp.tile([128, JB], dt.float32, name="pos_cols")
                lhs2 = znt_own[e][:, :, t * 128:(t + 1) * 128]  # [128, 2, 128] fp8
                for jb in range(JB):
                    sim_ps = psl.tile([128, JW], dt.float32, name="sim_ps",
                                      tag="sim", bufs=2)
                    for h in range(2):  # 512-wide halves: one PSUM bank per matmul
                        j0 = jb * JW + h * 512
                        # DoubleRow: virtual K=256 via fp8 pair-packing
                        nc.tensor.matmul(sim_ps[:, h * 512:(h + 1) * 512], lhs2,
                                         znt_sb[e][:, :, j0:j0 + 512],
                                         start=True, stop=True,
                                         perf_mode=mybir.MatmulPerfMode.DoubleRow)
                    refl = lp.tile([128, JW], dt.bfloat16, name="refl", bufs=3)
                    nc.scalar.activation(refl[:], sim_ps[:], AF.Exp,
                                         accum_out=tot_cols[:, jb:jb + 1])
                    adj_t = lp.tile([128, JW], dt.bfloat16, name="adj_t", bufs=4)
                    nc.sync.dma_start(adj_t[:], adj_in[akey][t, jb])
                    mscr = lp.tile([128, JW], dt.bfloat16, name="mscr", bufs=3)
                    nc.vector._custom_dve(
                        TENSOR_TENSOR_REDUCE, out=mscr[:], in0=refl[:],
                        in1=adj_t[:], s0=0.0, s1=1.0,
                        accum_out=pos_cols[:, jb:jb + 1])
                nc.vector.reduce_sum(tot_all[:, t:t + 1], tot_cols[:],
                                     axis=mybir.AxisListType.X)
                nc.vector.reduce_sum(pos_all[:, t:t + 1], pos_cols[:],
                                     axis=mybir.AxisListType.X)

            def loss_final(il, e):
                tot_all, pos_all = loss_stats[e]
                neg = lp.tile([128, NT], dt.float32, name="neg", bufs=1)
                nc.vector.tensor_tensor(out=neg[:], in0=tot_all[:], in1=pos_all[:],
                                        op=AL.subtract)
                if debug:
                    psd = work.tile([128, NT], dt.float32, name="psd")
                    nc.vector.tensor_copy(psd[:], pos_all[:])
                    nc.sync.dma_start(dbg["pt"][il, 0], psd[:])
                    ttd = work.tile([128, NT], dt.float32, name="ttd")
                    nc.vector.tensor_copy(ttd[:], tot_all[:])
                    nc.sync.dma_start(dbg["pt"][il, 1], ttd[:])
                nc.vector.tensor_scalar(out=pos_all[:], in0=pos_all[:],
                                        scalar1=SIGMA, scalar2=None, op0=AL.add)
                nc.vector.tensor_scalar(out=neg[:], in0=neg[:],
                                        scalar1=SIGMA, scalar2=None, op0=AL.add)
                nc.scalar.activation(pos_all[:], pos_all[:], AF.Ln)
                nc.scalar.activation(neg[:], neg[:], AF.Ln)
                dl = lp.tile([128, NT], dt.float32, name="dl", bufs=1)
                nc.vector.tensor_tensor(out=dl[:], in0=neg[:], in1=pos_all[:],
                                        op=AL.subtract)
                nc.vector.reduce_sum(loss_parts[:, il:il + 1], dl[:],
                                     axis=mybir.AxisListType.X)

            # =======================================================
            # issue schedule
            # =======================================================
            with tc.tile_pool(name="psg", bufs=1, space="PSUM") as psg, \
                 tc.tile_pool(name="psl", bufs=1, space="PSUM") as psl:

                # L1 both graphs; h1w all-gathers overlap the other graph
                with tc.tile_pool(name="l1p", bufs=2) as l1p:
                    for t in range(NT):
                        l1_tile("a", t)
                    nc.gpsimd.collective_compute(
                        "AllGather", AL.bypass, replica_groups=RG,
                        ins=[h1w_loc["a"][:]], outs=[h1w_full["a"][:]])
                    for t in range(NT):
                        l1_tile("x", t)
                    nc.gpsimd.collective_compute(
                        "AllGather", AL.bypass, replica_groups=RG,
                        ins=[h1w_loc["x"][:]], outs=[h1w_full["x"][:]])

                # separate scope so l1p's stream buffers are released before
                # the l2/loss pools allocate
                l2p_cm = tc.tile_pool(name="l2p", bufs=2)
                lp_cm = tc.tile_pool(name="loss", bufs=2)
                l2p = l2p_cm.__enter__()
                lp = lp_cm.__enter__()

                # L2-a -> h2a -> za -> AG
                for t in range(NT):
                    l2_tile("a", t)
                rsqrt_batch("za")
                z_tiles("za", h2_sb["a"])
                znt_load("za")

                # loss-a interleaved with L2-x
                for t in range(NT):
                    loss_tile("za", "label", t)
                    l2_tile("x", t)
                loss_final(0, "za")

                rsqrt_batch("zx")
                z_tiles("zx", h2_sb["x"])
                znt_load("zx")

                # ---------- attention fusion (tanh via exp) ----------
                w_rows = statp.tile([1, 2 * ROWS], dt.float32)  # wx | wadj
                for ib, g in enumerate(("x", "a")):
                    for t in range(NT):
                        t1_ps = psg.tile([16, 128], dt.float32, name="t1_ps",
                                         tag="acc", bufs=2)
                        for kc in range(2):
                            trh_ps = psg.tile([128, 128], dt.bfloat16, name="trh_ps",
                                              tag="tr", bufs=2)
                            nc.tensor.transpose(
                                trh_ps[:],
                                h2_sb[g][:, t * OUT + kc * 128: t * OUT + kc * 128 + 128],
                                idbf_sb[:])
                            trh = work.tile([128, 128], dt.bfloat16, name="trh")
                            nc.vector.tensor_copy(trh[:], trh_ps[:])
                            nc.tensor.matmul(t1_ps[:], wp1_sb[:, kc, :],
                                             trh[:], start=(kc == 0), stop=False)
                        nc.tensor.matmul(t1_ps[:], bp1_sb[:], ones_row[:],
                                         start=False, stop=True)
                        # tanh(u) = 1 - 2/(exp(2u)+1)
                        e2u = work.tile([16, 128], dt.float32, name="e2u")
                        nc.scalar.activation(e2u[:], t1_ps[:], AF.Exp, scale=2.0)
                        nc.vector.tensor_scalar(out=e2u[:], in0=e2u[:], scalar1=1.0,
                                                scalar2=None, op0=AL.add)
                        nc.vector.reciprocal(e2u[:], e2u[:])
                        t1_sb = work.tile([16, 128], dt.bfloat16, name="t1_sb")
                        nc.vector.tensor_scalar(out=t1_sb[:], in0=e2u[:], scalar1=-2.0,
                                                scalar2=1.0, op0=AL.mult, op1=AL.add)
                        w_ps = psg.tile([1, 128], dt.float32, name="w_ps",
                                        tag="acc", bufs=2)
                        nc.tensor.matmul(w_ps[:], wp2_sb[:], t1_sb[:], start=True, stop=True)
                        nc.vector.tensor_copy(
                            w_rows[:, ib * ROWS + t * 128: ib * ROWS + (t + 1) * 128],
                            w_ps[:])
                # beta_x = sigmoid(wx - wadj) on [1, 1024]
                dw = statp.tile([1, ROWS], dt.float32)
                nc.vector.tensor_tensor(out=dw[:], in0=w_rows[:, 0:ROWS],
                                        in1=w_rows[:, ROWS:2 * ROWS], op=AL.subtract)
                nc.scalar.activation(dw[:], dw[:], AF.Exp, scale=-1.0)
                nc.vector.tensor_scalar(out=dw[:], in0=dw[:], scalar1=1.0,
                                        scalar2=None, op0=AL.add)
                nc.vector.reciprocal(dw[:], dw[:])
                nc.sync.dma_start(dw_dram.rearrange("(o x) -> o x", o=1), dw[:])
                beta_col = statp.tile([128, 1, NT], dt.float32)
                nc.sync.dma_start(beta_col[:],
                                  dw_dram.rearrange("(t p o) -> p o t", p=128, o=1))
                if debug:
                    nc.sync.dma_start(dbg["beta"][:], beta_col[:, 0, :])
                # h_fuse = h_adj + beta*(h_x - h_adj); row norm^2 on the fly
                from concourse.dve_ops import TENSOR_TENSOR_REDUCE as TTR
                for t in range(NT):
                    dhf = work.tile([128, OUT], dt.bfloat16, name="dhf")
                    nc.vector.tensor_tensor(out=dhf[:], in0=h2_sb["x"][:, t * OUT:(t + 1) * OUT],
                                            in1=h2_sb["a"][:, t * OUT:(t + 1) * OUT],
                                            op=AL.subtract)
                    seg = hf_sb[:, t * OUT:(t + 1) * OUT]
                    nc.vector.scalar_tensor_tensor(
                        out=seg, in0=dhf[:],
                        scalar=beta_col[:, 0, t:t + 1],
                        in1=h2_sb["a"][:, t * OUT:(t + 1) * OUT],
                        op0=AL.mult, op1=AL.add)
                    scr = work.tile([128, OUT], dt.bfloat16, name="scr")
                    nc.vector._custom_dve(TTR, out=scr[:], in0=seg, in1=seg,
                                          s0=0.0, s1=1.0,
                                          accum_out=nrm2["zf"][:, t:t + 1])
                if debug:
                    for t in range(NT):
                        hfd = work.tile([128, OUT], dt.float32, name="hfd")
                        nc.vector.tensor_copy(hfd[:], hf_sb[:, t * OUT:(t + 1) * OUT])
                        nc.sync.dma_start(dbg["hf"][t * 128:(t + 1) * 128, :], hfd[:])

                rsqrt_batch("zf")
                z_tiles("zf", hf_sb)
                znt_load("zf")

                # ---------- dim partials + AllReduce (overlaps loss-x) ----------
                # colsum(X) is input-static: computed host-side (csum_in).
                hfb = lp.tile([128, NT, OUT], dt.bfloat16, bufs=1)
                for t in range(NT):
                    nc.vector.tensor_copy(hfb[:, t, :], hf_sb[:, t * OUT:(t + 1) * OUT])
                dim_sb = lp.tile([128, 4, OUT], dt.float32, bufs=1)
                for mt in range(4):
                    xtz_ps = psg.tile([128, OUT], dt.float32, name="xtz_ps",
                                      tag="acc", bufs=2)
                    for t in range(NT):
                        nc.tensor.matmul(xtz_ps[:],
                                         xblk_sb[:, t, mt * 128:(mt + 1) * 128],
                                         hfb[:, t, :], start=(t == 0), stop=(t == NT - 1))
                    nc.vector.tensor_copy(dim_sb[:, mt, :], xtz_ps[:])
                nc.sync.dma_start(dim_loc.rearrange("m p f -> p m f"), dim_sb[:])
                nc.gpsimd.collective_compute(
                    "AllReduce", AL.add, replica_groups=RG,
                    ins=[dim_loc[:]], outs=[dim_full[:]])

                # ---------- loss-x ----------
                for t in range(NT):
                    loss_tile("zx", "X", t)
                loss_final(1, "zx")

                # ---------- dim centers ----------
                dimf = lp.tile([128, 4, OUT], dt.float32, bufs=1)
                nc.sync.dma_start(dimf[:], dim_full.rearrange("m p f -> p m f"))
                dcnT = lp.tile([128, 2, 512], dt.float8e4, bufs=1)
                dcn2 = lp.tile([128, 4], dt.float32, bufs=1)
                dc_store = lp.tile([128, 4, OUT], dt.bfloat16, bufs=1)
                for mt in range(4):
                    nc.vector.tensor_scalar(out=dc_store[:, mt, :], in0=dimf[:, mt, :],
                                            scalar1=csum_sb[:, mt:mt + 1],
                                            scalar2=None, op0=AL.mult)
                    if debug:
                        dcd = work.tile([128, OUT], dt.float32, name="dcd")
                        nc.vector.tensor_copy(dcd[:], dc_store[:, mt, :])
                        nc.sync.dma_start(dbg["dc"][mt], dcd[:])
                    scr = work.tile([128, OUT], dt.bfloat16, name="scrd")
                    nc.vector._custom_dve(TENSOR_TENSOR_REDUCE, out=scr[:],
                                          in0=dc_store[:, mt, :], in1=dc_store[:, mt, :],
                                          s0=0.0, s1=1.0, accum_out=dcn2[:, mt:mt + 1])
                nc.vector.tensor_scalar(out=dcn2[:], in0=dcn2[:], scalar1=1e-30,
                                        scalar2=None, op0=AL.max)
                nc.scalar.activation(dcn2[:], dcn2[:], AF.Ln)
                nc.scalar.activation(dcn2[:], dcn2[:], AF.Exp, scale=-0.5)
                for mt in range(4):
                    dcn_t = work.tile([128, OUT], dt.bfloat16, name="dcn_t")
                    nc.vector.tensor_scalar(out=dcn_t[:], in0=dc_store[:, mt, :],
                                            scalar1=dcn2[:, mt:mt + 1],
                                            scalar2=None, op0=AL.mult)
                    for kc in range(2):
                        dct_ps = psg.tile([128, 128], dt.bfloat16, name="dct_ps",
                                          tag="tr", bufs=2)
                        nc.tensor.transpose(dct_ps[:], dcn_t[:, kc * 128:(kc + 1) * 128],
                                            idbf_sb[:])
                        nc.vector.tensor_copy(dcnT[:, kc, mt * 128:(mt + 1) * 128],
                                              dct_ps[:])

                # ---------- loss-f ----------
                for t in range(NT):
                    loss_tile("zf", "rec", t)
                loss_final(2, "zf")

                # ---------- dim-label loss ----------
                tot2 = lp.tile([128, NT], dt.float32, bufs=1)
                pos2 = lp.tile([128, NT], dt.float32, bufs=1)
                for t in range(NT):
                    r2_ps = psl.tile([128, 512], dt.float32, name="r2_ps",
                                     tag="sim", bufs=2)
                    nc.tensor.matmul(r2_ps[:], znt_own["zf"][:, :, t * 128:(t + 1) * 128],
                                     dcnT[:, :, :], start=True, stop=True,
                                     perf_mode=mybir.MatmulPerfMode.DoubleRow)
                    refl2 = lp.tile([128, 512], dt.bfloat16, name="refl2")
                    nc.scalar.activation(refl2[:], r2_ps[:], AF.Exp,
                                         accum_out=tot2[:, t:t + 1])
                    xhot = lp.tile([128, 512], dt.bfloat16, name="xhot")
                    nc.vector.tensor_scalar(out=xhot[:], in0=xblk_sb[:, t, :],
                                            scalar1=0.0, scalar2=None, op0=AL.is_gt)
                    scr2 = lp.tile([128, 512], dt.bfloat16, name="scr2")
                    nc.vector._custom_dve(TENSOR_TENSOR_REDUCE, out=scr2[:],
                                          in0=refl2[:], in1=xhot[:], s0=0.0, s1=1.0,
                                          accum_out=pos2[:, t:t + 1])
                if debug:
                    p2d = work.tile([128, NT], dt.float32, name="p2d")
                    nc.vector.tensor_copy(p2d[:], pos2[:])
                    nc.sync.dma_start(dbg["pt2"][0], p2d[:])
                    t2d = work.tile([128, NT], dt.float32, name="t2d")
                    nc.vector.tensor_copy(t2d[:], tot2[:])
                    nc.sync.dma_start(dbg["pt2"][1], t2d[:])
                # loss_feat partial: -ln(pos/neg + 1e-5), pos=pos2+SIG, neg=tot2-pos2
                neg2 = lp.tile([128, NT], dt.float32, bufs=1)
                nc.vector.tensor_tensor(out=neg2[:], in0=tot2[:], in1=pos2[:],
                                        op=AL.subtract)
                nc.vector.tensor_scalar(out=pos2[:], in0=pos2[:], scalar1=SIGMA,
                                        scalar2=None, op0=AL.add)
                nc.vector.reciprocal(neg2[:], neg2[:])
                r = lp.tile([128, NT], dt.float32, bufs=1)
                nc.vector.tensor_tensor(out=r[:], in0=pos2[:], in1=neg2[:], op=AL.mult)
                nc.vector.tensor_scalar(out=r[:], in0=r[:], scalar1=1e-5,
                                        scalar2=None, op0=AL.add)
                nc.scalar.activation(r[:], r[:], AF.Ln)
                rsum = lp.tile([128, 1], dt.float32, bufs=1)
                nc.vector.reduce_sum(rsum[:], r[:], axis=mybir.AxisListType.X)
                nc.vector.tensor_scalar(out=loss_parts[:, 3:4], in0=rsum[:],
                                        scalar1=-1.0, scalar2=None, op0=AL.mult)

                lp_cm.__exit__(None, None, None)
                l2p_cm.__exit__(None, None, None)

            # ---------- output ----------
            nc.sync.dma_start(out_t[:], loss_parts[:])

    nc.compile()
    return nc


# ---------------------------------------------------------------- entry point
def _prep(feat, adj_label, adj_X, adj_rec, W0a, b0a, W1a, b1a,
          W0x, b0x, W1x, b1x, Wp1, bp1, wp2, edge_index, edge_index_x,
          _debug=False):
    feat = np.asarray(feat, np.float32)
    feat_bf = feat.astype(BF16)
    ga = _prep_graph(np.asarray(edge_index), feat_bf)
    gx = _prep_graph(np.asarray(edge_index_x), feat_bf)

    key = (ga["nb_d"], gx["nb_d"], _debug)
    if key not in _cache:
        _cache[key] = _build(*key[:2], debug=_debug)
    nc = _cache[key]

    idbf = np.eye(128, dtype=np.float32).astype(BF16)

    colsum = feat.sum(axis=0)  # [IN]
    crecip = (1.0 / (colsum + 1e-5)).astype(np.float32).reshape(4, 128).T

    base = dict(
        idbf=idbf,
        csum=np.ascontiguousarray(crecip),
        W0a=np.asarray(W0a, np.float32).astype(BF16),
        W1a=np.asarray(W1a, np.float32).astype(BF16),
        b0a=np.asarray(b0a, np.float32).reshape(1, HID).astype(BF16),
        b1a=np.asarray(b1a, np.float32).reshape(1, OUT).astype(BF16),
        W0x=np.asarray(W0x, np.float32).astype(BF16),
        W1x=np.asarray(W1x, np.float32).astype(BF16),
        b0x=np.asarray(b0x, np.float32).reshape(1, HID).astype(BF16),
        b1x=np.asarray(b1x, np.float32).reshape(1, OUT).astype(BF16),
        Wp1=np.asarray(Wp1, np.float32).astype(BF16),
        bp1=np.asarray(bp1, np.float32).reshape(1, ATT_H).astype(BF16),
        wp2=np.asarray(wp2, np.float32).astype(BF16),
    )
    adj_bf = {k: np.asarray(v, np.float32).astype(BF16)
              for k, v in (("label", adj_label), ("X", adj_X), ("rec", adj_rec))}

    in_maps = []
    for c in range(NC_):
        m = dict(base)
        m["xblk"] = feat_bf[c * ROWS:(c + 1) * ROWS]
        for k in ("label", "X", "rec"):
            blk = adj_bf[k][c * ROWS:(c + 1) * ROWS]  # [1024, 8192]
            m[f"adj_{k}"] = np.ascontiguousarray(
                blk.reshape(NT, 128, JB, JW).transpose(0, 2, 1, 3))
        for gname, g in (("a", ga), ("x", gx)):
            m[f"gfeat_{gname}"] = g["gfeat"][c]
            m[f"sblk_{gname}"] = g["sblk"][c]
            m[f"srcidx_{gname}"] = np.ascontiguousarray(g["src_idx"][c])
            m[f"nd_{gname}"] = g["nd_arr"][c]
        in_maps.append(m)

    return nc, in_maps


def kernel(_debug=False, _trace=False, **inputs):
    from concourse.bass_utils import run_bass_kernel_spmd
    nc, in_maps = _prep(_debug=_debug, **inputs)
    res = run_bass_kernel_spmd(nc, in_maps, core_ids=list(range(NC_)), trace=_trace)
    parts = np.stack([r["out"] for r in res.results])  # [8, 128, 8]
    psum = parts.sum(axis=(0, 1))  # [8]
    la, lx, ladj, lf = psum[0] / N, psum[1] / N, psum[2] / N, psum[3] / N
    val = np.float32(LAM * (la + lx) + ALPHA * lf + ladj)
    if _debug or _trace:
        kernel._last = res
    return np.asarray(val, np.float32).reshape(())
